# revision 30
# baseline (speedup 1.0000x reference)
"""KNN retrieval kernel for Trainium2 (8 NeuronCores, data-parallel over queries).

Problem: for each query row x[i] (N=16384, DIM=16), find j* = argmin_j ||xb[j]-x[i]||
over M=16384 reference rows and return y[j*].

Device algorithm (per core, 2048 queries; ALGO="pairscan"):
  ms[i,j] = 2<x_i, xb_j> - ||xb_j||^2   (argmax_j ms == argmin_j dist; the
            ||x_i||^2 term is constant per row and dropped)
  - PE: ms computed as K=50 split-bf16 matmuls (fp32-accurate), 2 j-tiles
    packed into the 128x128 array via 64-row groups, 4 matmuls per 2048-wide
    PSUM chunk.
  - ACT: copies each even chunk PSUM->SBUF (absorbing half the PSUM reads).
  - DVE: chained tensor_tensor_scan(op0=max, op1=max) over (odd-chunk PSUM,
    copied-even-chunk SBUF) ingests TWO score streams per instruction while
    building the row prefix-max P in j order (host packs column q of chunk
    2m+s as ref j = 4096m+2q+s, so scan position K covers pair {2K, 2K+1}).
  - ACT: Sign+accum counts #{k: P[16k+15] < g} (g = last prefix) = the index
    of the first 32-wide j-window containing the row max; exact on ties
    since the prefix is monotone and pairs are j-ordered.
  - GPSIMD: indirect-DMA gathers that window's 32 augmented xb columns; DVE
    re-dots them in exact fp32 and max8/max_index picks the first argmax;
    j* = 32*b* + pos; GPSIMD gathers y[j*].
Host: builds augmented/packed layouts, shards queries 8 ways, reassembles.
"""

import os
import sys

sys.path.insert(0, "/opt/trn_rl_repo")

import numpy as np

N, M, DIM = 16384, 16384, 16
NCORES = 8
NQ = N // NCORES  # queries per core
RB = 128          # row-block (queries per partition block)
JT = 512          # j-tile width (one PSUM bank of fp32)
TPG = 4           # j-tiles packed per PE group (32-row groups)
CHUNK = TPG * JT  # scan chunk width (4 PSUM banks)
K_AUG = 17        # 16 dims + 1 augmentation row
K_SPL = 50        # bf16-split contraction: 16 hi + 2 aug + 16 lo + 16 hi


WSUB = 32         # sub-block width for the submax algorithm


def build_nc(nq=NQ, m=M, mode="fp32", loop_n=0, parts="full",
             count_engine="act", algo="scan", n_act=6,
             pair_both_copy=False):
    """Build the per-core Bass module. loop_n>0 wraps the compute in a
    hardware repeat loop (for timing measurement only). parts in
    {"full", "mm", "mmscan"} selects pipeline stages (for perf bisection)."""
    import contextlib
    from contextlib import ExitStack

    import concourse.bacc as bacc
    import concourse.bass as bass
    import concourse.mybir as mybir
    import concourse.tile as tile
    from concourse.bass import IndirectOffsetOnAxis

    fp32 = mybir.dt.float32
    n_rb = nq // RB
    n_chunk = m // CHUNK
    NEGINF = float(np.float32(-3.0e38))

    nc = bacc.Bacc("TRN2", target_bir_lowering=False, debug=False)

    in_dt = mybir.dt.bfloat16 if mode == "bf16split" else fp32
    xb_free = n_chunk * (2 if mode == "bf16split" else TPG) * JT
    xq_d = nc.dram_tensor("xq4", [128, nq], in_dt, kind="ExternalInput")
    xb_d = nc.dram_tensor("xbp", [128, xb_free], in_dt, kind="ExternalInput")
    y_d = nc.dram_tensor("ytab", [m, 1], fp32, kind="ExternalInput")
    out_d = nc.dram_tensor("yout", [128, n_rb], fp32, kind="ExternalOutput")
    if algo in ("submax", "submax16", "pairscan", "countmax"):
        xw_d = nc.dram_tensor("xw", [m // WSUB, K_AUG * WSUB], fp32,
                              kind="ExternalInput")
        xqr_d = nc.dram_tensor("xqr", [128, n_rb * K_AUG], fp32,
                               kind="ExternalInput")

    with tile.TileContext(nc) as tc:
        with ExitStack() as ctx:
            consts = ctx.enter_context(tc.tile_pool(name="consts", bufs=1))
            psum_pool = ctx.enter_context(
                tc.tile_pool(name="ps", bufs=2, space=bass.MemorySpace.PSUM))
            pms_pool = ctx.enter_context(tc.tile_pool(name="pms", bufs=3))
            gpool = ctx.enter_context(tc.tile_pool(name="g", bufs=3))
            outp = ctx.enter_context(tc.tile_pool(name="outp", bufs=1))

            assert n_chunk % 2 == 0
            half_chunks = n_chunk // 2
            half = half_chunks * CHUNK

            xq4 = consts.tile([128, nq], in_dt)
            xb = consts.tile([128, xb_free], in_dt)
            nc.sync.dma_start(xq4[:], xq_d[:])
            nc.sync.dma_start(xb[:], xb_d[:])
            if mode == "bf16split":
                dummy = consts.tile([128, CHUNK], fp32)
                nc.vector.memset(dummy[:], 0.0)

            J0 = outp.tile([128, n_rb], fp32)
            J1 = outp.tile([128, n_rb], fp32)
            Yg = outp.tile([128, n_rb], fp32)
            if parts != "full":
                nc.gpsimd.memset(Yg[:], 0.0)

            def emit_mms(rb, t, ps):
                if mode == "bf16split":
                    # K=50 split-bf16 contraction, 2-way row packing
                    for v in range(2):
                        for s in range(2):
                            u = 2 * v + s
                            nc.tensor.matmul(
                                ps[:, u * JT:(u + 1) * JT],
                                xq4[64 * s:64 * s + K_SPL,
                                    rb * RB:(rb + 1) * RB],
                                xb[64 * s:64 * s + K_SPL,
                                   (t * 2 + v) * JT:(t * 2 + v + 1) * JT],
                                start=True,
                                stop=True,
                                tile_position=(64 * s, 0),
                            )
                else:
                    for b in range(TPG):
                        nc.tensor.matmul(
                            ps[:, b * JT:(b + 1) * JT],
                            xq4[32 * b:32 * b + K_AUG,
                                rb * RB:(rb + 1) * RB],
                            xb[32 * b:32 * b + K_AUG,
                               (t * TPG + b) * JT:(t * TPG + b + 1) * JT],
                            start=True,
                            stop=True,
                            tile_position=(32 * b, 0),
                        )

            if algo in ("submax", "submax16", "pairscan", "countmax"):
                xqr = consts.tile([128, n_rb * K_AUG], fp32)
                nc.sync.dma_start(xqr[:], xqr_d[:])
                smpool = ctx.enter_context(tc.tile_pool(name="sm", bufs=2))
                wpool = ctx.enter_context(tc.tile_pool(name="w", bufs=3))
                nsub = m // WSUB
                cps = CHUNK // WSUB  # sub-blocks per chunk
                WK = WSUB * K_AUG
            if algo in ("submax16", "pairscan"):
                r16pool = ctx.enter_context(tc.tile_pool(name="r16", bufs=2))

            def emit_countmax_rb(rb):
                """submax main loop + count-based tail: per-chunk TR gives
                block maxima SM; a tiny prefix-max scan over SM plus an ACT
                sign-count locate the first block achieving the row max;
                one exact fp32 window re-dot resolves j*."""
                SM = smpool.tile([128, nsub], fp32, name=f"cm{rb}", tag="sm")
                PS = smpool.tile([128, nsub], fp32, name=f"cp{rb}", tag="px")
                for t in range(n_chunk):
                    ps = psum_pool.tile([128, CHUNK], fp32, name=f"p{rb}_{t}",
                                        tag="ps")
                    emit_mms(rb, t, ps)
                    if parts == "mm":
                        nc.vector.tensor_copy(SM[:, t * cps:t * cps + 8],
                                              ps[:, 0:8])
                        continue
                    nc.vector.tensor_reduce(
                        SM[:, t * cps:(t + 1) * cps],
                        ps[:].rearrange("p (s w) -> p s w", w=WSUB),
                        mybir.AxisListType.X,
                        mybir.AluOpType.max,
                    )
                if parts != "full":
                    return
                nc.vector.tensor_tensor_scan(
                    PS[:], SM[:], SM[:], NEGINF,
                    mybir.AluOpType.max, mybir.AluOpType.bypass)
                g = PS[:, nsub - 1:nsub]
                cnt = gpool.tile([128, 1], fp32)
                sj = gpool.tile([128, nsub], fp32, name=f"sj{rb}", tag="sj")
                nc.scalar.activation(
                    sj[:], PS[:], mybir.ActivationFunctionType.Sign,
                    bias=g, scale=-1.0, accum_out=cnt[:])
                bidx = gpool.tile([128, 1], mybir.dt.uint32)
                nc.vector.tensor_copy(bidx[:], cnt[:])
                Wt = wpool.tile([128, WK], fp32)
                nc.gpsimd.indirect_dma_start(
                    Wt[:], None, xw_d[:], IndirectOffsetOnAxis(bidx[:], 0))
                xq_b = (xqr[:, rb * K_AUG:(rb + 1) * K_AUG]
                        .rearrange("p (c k) -> p c k", c=1)
                        .to_broadcast([128, WSUB, K_AUG]))
                Dt = wpool.tile([128, WK], fp32)
                Dd = wpool.tile([128, WSUB], fp32)
                dt_v = Dt[:].rearrange("p (c k) -> p c k", k=K_AUG)
                nc.vector.tensor_tensor(
                    dt_v, Wt[:].rearrange("p (k c) -> p c k", c=WSUB),
                    xq_b, op=mybir.AluOpType.mult)
                nc.vector.tensor_reduce(
                    Dd[:], dt_v, mybir.AxisListType.X, mybir.AluOpType.add)
                cm8 = gpool.tile([128, 8], fp32)
                ci8 = gpool.tile([128, 8], mybir.dt.uint32)
                nc.vector.max(cm8[:], Dd[:])
                nc.vector.max_index(ci8[:], cm8[:], Dd[:])
                c2f = gpool.tile([128, 1], fp32)
                jf = gpool.tile([128, 1], fp32)
                ji = gpool.tile([128, 1], mybir.dt.uint32)
                nc.vector.tensor_copy(c2f[:], ci8[:, 0:1])
                nc.vector.scalar_tensor_tensor(
                    jf[:], cnt[:], float(WSUB), c2f[:],
                    mybir.AluOpType.mult, mybir.AluOpType.add)
                nc.vector.tensor_copy(ji[:], jf[:])
                nc.gpsimd.indirect_dma_start(
                    Yg[:, rb:rb + 1], None, y_d[:],
                    IndirectOffsetOnAxis(ap=ji[:], axis=0))

            def emit_pairscan_rb(rb):
                """Pair-scan: ACT copies even chunks PSUM->SBUF; DVE runs a
                chained tensor_tensor_scan with op0=op1=max over (odd-chunk
                PSUM, copied-even-chunk SBUF) pairs, ingesting TWO score
                streams per cycle while building the row prefix-max in j
                order (host packs column c of chunk 2m+s as ref j =
                4096m+2c+s, so pair index = j//2). The decimated count
                #{P[16k+15] < g} is the 32-wide window holding the first
                max, resolved exactly by one window gather + fp32 re-dot."""
                npair = n_chunk // 2
                P = smpool.tile([128, m // 2], fp32, name=f"pp{rb}",
                                tag="pp")
                for mm_ in range(npair):
                    Ca = r16pool.tile([128, CHUNK], fp32)
                    psA = psum_pool.tile([128, CHUNK], fp32,
                                         name=f"pA{rb}_{mm_}", tag="ps")
                    emit_mms(rb, 2 * mm_, psA)
                    psB = psum_pool.tile([128, CHUNK], fp32,
                                         name=f"pB{rb}_{mm_}", tag="ps")
                    emit_mms(rb, 2 * mm_ + 1, psB)
                    if parts == "mm":
                        nc.vector.tensor_copy(
                            P[:, mm_ * CHUNK:mm_ * CHUNK + 8], psA[:, 0:8])
                        nc.vector.tensor_copy(
                            P[:, mm_ * CHUNK + 8:mm_ * CHUNK + 16],
                            psB[:, 0:8])
                        continue
                    nc.scalar.copy(Ca[:], psA[:])
                    initial = (NEGINF if mm_ == 0
                               else P[:, mm_ * CHUNK - 1:mm_ * CHUNK])
                    if pair_both_copy:
                        Cb = r16pool.tile([128, CHUNK], fp32)
                        nc.scalar.copy(Cb[:], psB[:])
                        data0 = Cb
                    else:
                        data0 = psB
                    nc.vector.tensor_tensor_scan(
                        P[:, mm_ * CHUNK:(mm_ + 1) * CHUNK],
                        data0[:],
                        Ca[:],
                        initial,
                        mybir.AluOpType.max,
                        mybir.AluOpType.max,
                    )
                if parts != "full":
                    return None
                g = P[:, m // 2 - 1:m // 2]
                # window index b* = #{k: P[16k+15] < g} via ACT sign-count
                cnt = gpool.tile([128, 1], fp32)
                sj = gpool.tile([128, 512], fp32, name=f"sj{rb}", tag="sj")
                nc.scalar.activation(
                    sj[:].rearrange("p (k s) -> p k s", s=1),
                    P[:].rearrange("p (k s) -> p k s", s=16)[:, :, 15:16],
                    mybir.ActivationFunctionType.Sign,
                    bias=g,
                    scale=-1.0,
                    accum_out=cnt[:],
                )
                return (rb, cnt)

            def emit_pairscan_mid(pend):
                """Stage 2: b* -> uint32 and window-gather launch. Emitted one
                rb after the scan so the ACT count has already landed."""
                rb, cnt = pend
                bidx = gpool.tile([128, 1], mybir.dt.uint32)
                nc.vector.tensor_copy(bidx[:], cnt[:])
                Wt = wpool.tile([128, WK], fp32)
                nc.gpsimd.indirect_dma_start(
                    Wt[:], None, xw_d[:], IndirectOffsetOnAxis(bidx[:], 0))
                return (rb, cnt, Wt)

            def emit_pairscan_tail(pend):
                """Re-dot tail for a previous rb, emitted after the next rb's
                scan chain so the DVE never stalls on the count/gather."""
                rb, cnt, Wt = pend
                # exact fp32 re-dot of the 32-wide window
                xq_b = (xqr[:, rb * K_AUG:(rb + 1) * K_AUG]
                        .rearrange("p (c k) -> p c k", c=1)
                        .to_broadcast([128, WSUB, K_AUG]))
                Dt = wpool.tile([128, WK], fp32)
                Dd = wpool.tile([128, WSUB], fp32)
                dt_v = Dt[:].rearrange("p (c k) -> p c k", k=K_AUG)
                nc.vector.tensor_tensor(
                    dt_v, Wt[:].rearrange("p (k c) -> p c k", c=WSUB),
                    xq_b, op=mybir.AluOpType.mult)
                nc.vector.tensor_reduce(
                    Dd[:], dt_v, mybir.AxisListType.X, mybir.AluOpType.add)
                cm8 = gpool.tile([128, 8], fp32)
                ci8 = gpool.tile([128, 8], mybir.dt.uint32)
                nc.vector.max(cm8[:], Dd[:])
                nc.vector.max_index(ci8[:], cm8[:], Dd[:])
                # j* = 32*b* + pos
                c2f = gpool.tile([128, 1], fp32)
                jf = gpool.tile([128, 1], fp32)
                ji = gpool.tile([128, 1], mybir.dt.uint32)
                nc.vector.tensor_copy(c2f[:], ci8[:, 0:1])
                nc.vector.scalar_tensor_tensor(
                    jf[:], cnt[:], float(WSUB), c2f[:],
                    mybir.AluOpType.mult, mybir.AluOpType.add)
                nc.vector.tensor_copy(ji[:], jf[:])
                nc.gpsimd.indirect_dma_start(
                    Yg[:, rb:rb + 1], None, y_d[:],
                    IndirectOffsetOnAxis(ap=ji[:], axis=0))

            def emit_submax16_rb(rb, n_act):
                """fp16 submax: per chunk, block maxima are computed in fp16
                (monotone rounding keeps the true argmax block tied-at-top;
                the exact fp32 re-dot of the top-2 windows resolves order, and
                max_index returns successive indices for tied values so both
                tied blocks are examined). Chunks < n_act go through an ACT
                copy PSUM->SBUF fp16 + DVE 2x tensor_reduce; the rest use a
                DVE tensor_tensor(max) pairing read straight from PSUM."""
                fp16 = mybir.dt.float16
                cps16 = CHUNK // WSUB  # blocks per chunk (64)
                SM = smpool.tile([128, nsub], fp16, name=f"sm16_{rb}")
                for t in range(n_chunk):
                    ps = psum_pool.tile([128, CHUNK], fp32, name=f"p{rb}_{t}",
                                        tag="ps")
                    emit_mms(rb, t, ps)
                    if parts == "mm":
                        nc.vector.tensor_copy(SM[:, t * cps16:t * cps16 + 8],
                                              ps[:, 0:8])
                        continue
                    if t < n_act:
                        R16 = r16pool.tile([128, CHUNK], fp16)
                        nc.scalar.copy(R16[:], ps[:])
                        nc.vector.tensor_reduce(
                            SM[:, t * cps16:(t + 1) * cps16],
                            R16[:].rearrange("p (s w) -> p s w", w=WSUB),
                            mybir.AxisListType.X,
                            mybir.AluOpType.max,
                        )
                    else:
                        # direct fp32 PSUM reduce (DVE 1x); out cast to fp16
                        nc.vector.tensor_reduce(
                            SM[:, t * cps16:(t + 1) * cps16],
                            ps[:].rearrange("p (s w) -> p s w", w=WSUB),
                            mybir.AxisListType.X,
                            mybir.AluOpType.max,
                        )
                if parts != "full":
                    return
                m8 = gpool.tile([128, 8], fp16)
                i8 = gpool.tile([128, 8], mybir.dt.uint32)
                nc.vector.max(m8[:], SM[:])
                nc.vector.max_index(i8[:], m8[:], SM[:])
                emit_redot_tail(rb, i8)

            def emit_redot_tail(rb, i8):
                """Exact fp32 re-dot of the top-2 candidate windows, then
                j* -> gather y[j*]. Shared by submax/submax16."""
                slo = gpool.tile([128, 1], mybir.dt.uint32)
                shi = gpool.tile([128, 1], mybir.dt.uint32)
                nc.vector.tensor_tensor(slo[:], i8[:, 0:1], i8[:, 1:2],
                                        op=mybir.AluOpType.min)
                nc.vector.tensor_tensor(shi[:], i8[:, 0:1], i8[:, 1:2],
                                        op=mybir.AluOpType.max)
                Wlo = wpool.tile([128, WK], fp32)
                Whi = wpool.tile([128, WK], fp32)
                nc.gpsimd.indirect_dma_start(
                    Wlo[:], None, xw_d[:], IndirectOffsetOnAxis(slo[:], 0))
                nc.gpsimd.indirect_dma_start(
                    Whi[:], None, xw_d[:], IndirectOffsetOnAxis(shi[:], 0))
                # exact fp32 re-dot of the two candidate windows
                xq_b = (xqr[:, rb * K_AUG:(rb + 1) * K_AUG]
                        .rearrange("p (c k) -> p c k", c=1)
                        .to_broadcast([128, WSUB, K_AUG]))
                Dt = wpool.tile([128, 2 * WK], fp32)
                Dd = wpool.tile([128, 2 * WSUB], fp32)
                for wi, Wt in ((0, Wlo), (1, Whi)):
                    dt_v = Dt[:, wi * WK:(wi + 1) * WK].rearrange(
                        "p (c k) -> p c k", k=K_AUG)
                    nc.vector.tensor_tensor(
                        dt_v, Wt[:].rearrange("p (k c) -> p c k", c=WSUB),
                        xq_b, op=mybir.AluOpType.mult)
                    nc.vector.tensor_reduce(
                        Dd[:, wi * WSUB:(wi + 1) * WSUB], dt_v,
                        mybir.AxisListType.X, mybir.AluOpType.add)
                cm8 = gpool.tile([128, 8], fp32)
                ci8 = gpool.tile([128, 8], mybir.dt.uint32)
                nc.vector.max(cm8[:], Dd[:])
                nc.vector.max_index(ci8[:], cm8[:], Dd[:])
                # j* = (c2<W ? slo : shi)*W + c2 mod W, all in fp32
                c2f = gpool.tile([128, 1], fp32)
                slof = gpool.tile([128, 1], fp32)
                shif = gpool.tile([128, 1], fp32)
                ge = gpool.tile([128, 1], fp32)
                t1 = gpool.tile([128, 1], fp32)
                jf = gpool.tile([128, 1], fp32)
                nc.vector.tensor_copy(c2f[:], ci8[:, 0:1])
                nc.vector.tensor_copy(slof[:], slo[:])
                nc.vector.tensor_copy(shif[:], shi[:])
                nc.vector.tensor_scalar(
                    out=ge[:], in0=c2f[:], scalar1=float(WSUB), scalar2=None,
                    op0=mybir.AluOpType.is_ge)
                nc.vector.tensor_sub(t1[:], shif[:], slof[:])
                nc.vector.tensor_mul(t1[:], ge[:], t1[:])
                nc.vector.tensor_add(t1[:], slof[:], t1[:])  # chosen s
                nc.vector.scalar_tensor_tensor(
                    jf[:], t1[:], float(WSUB), c2f[:],
                    mybir.AluOpType.mult, mybir.AluOpType.add)
                nc.vector.scalar_tensor_tensor(
                    jf[:], ge[:], float(-WSUB), jf[:],
                    mybir.AluOpType.mult, mybir.AluOpType.add)
                ji = gpool.tile([128, 1], mybir.dt.uint32)
                nc.vector.tensor_copy(ji[:], jf[:])
                nc.gpsimd.indirect_dma_start(
                    Yg[:, rb:rb + 1], None, y_d[:],
                    IndirectOffsetOnAxis(ap=ji[:], axis=0))

            def emit_submax_rb(rb):
                SM = smpool.tile([128, nsub], fp32)
                for t in range(n_chunk):
                    ps = psum_pool.tile([128, CHUNK], fp32, name=f"p{rb}_{t}",
                                        tag="ps")
                    emit_mms(rb, t, ps)
                    if parts == "mm":
                        nc.vector.tensor_copy(SM[:, t * cps:t * cps + 8],
                                              ps[:, 0:8])
                        continue
                    nc.vector.tensor_reduce(
                        SM[:, t * cps:(t + 1) * cps],
                        ps[:].rearrange("p (s w) -> p s w", w=WSUB),
                        mybir.AxisListType.X,
                        mybir.AluOpType.max,
                    )
                if parts != "full":
                    return
                # top-2 sub-blocks (value-ranked, then j-ordered)
                m8 = gpool.tile([128, 8], fp32)
                i8 = gpool.tile([128, 8], mybir.dt.uint32)
                nc.vector.max(m8[:], SM[:])
                nc.vector.max_index(i8[:], m8[:], SM[:])
                slo = gpool.tile([128, 1], mybir.dt.uint32)
                shi = gpool.tile([128, 1], mybir.dt.uint32)
                nc.vector.tensor_tensor(slo[:], i8[:, 0:1], i8[:, 1:2],
                                        op=mybir.AluOpType.min)
                nc.vector.tensor_tensor(shi[:], i8[:, 0:1], i8[:, 1:2],
                                        op=mybir.AluOpType.max)
                Wlo = wpool.tile([128, WK], fp32)
                Whi = wpool.tile([128, WK], fp32)
                nc.gpsimd.indirect_dma_start(
                    Wlo[:], None, xw_d[:], IndirectOffsetOnAxis(slo[:], 0))
                nc.gpsimd.indirect_dma_start(
                    Whi[:], None, xw_d[:], IndirectOffsetOnAxis(shi[:], 0))
                # exact fp32 re-dot of the two candidate windows
                xq_b = (xqr[:, rb * K_AUG:(rb + 1) * K_AUG]
                        .rearrange("p (c k) -> p c k", c=1)
                        .to_broadcast([128, WSUB, K_AUG]))
                Dt = wpool.tile([128, 2 * WK], fp32)
                Dd = wpool.tile([128, 2 * WSUB], fp32)
                for wi, Wt in ((0, Wlo), (1, Whi)):
                    dt_v = Dt[:, wi * WK:(wi + 1) * WK].rearrange(
                        "p (c k) -> p c k", k=K_AUG)
                    nc.vector.tensor_tensor(
                        dt_v, Wt[:].rearrange("p (k c) -> p c k", c=WSUB),
                        xq_b, op=mybir.AluOpType.mult)
                    nc.vector.tensor_reduce(
                        Dd[:, wi * WSUB:(wi + 1) * WSUB], dt_v,
                        mybir.AxisListType.X, mybir.AluOpType.add)
                cm8 = gpool.tile([128, 8], fp32)
                ci8 = gpool.tile([128, 8], mybir.dt.uint32)
                nc.vector.max(cm8[:], Dd[:])
                nc.vector.max_index(ci8[:], cm8[:], Dd[:])
                # j* = (c2<W ? slo : shi)*W + c2 mod W, all in fp32
                c2f = gpool.tile([128, 1], fp32)
                slof = gpool.tile([128, 1], fp32)
                shif = gpool.tile([128, 1], fp32)
                ge = gpool.tile([128, 1], fp32)
                t1 = gpool.tile([128, 1], fp32)
                jf = gpool.tile([128, 1], fp32)
                nc.vector.tensor_copy(c2f[:], ci8[:, 0:1])
                nc.vector.tensor_copy(slof[:], slo[:])
                nc.vector.tensor_copy(shif[:], shi[:])
                nc.vector.tensor_scalar(
                    out=ge[:], in0=c2f[:], scalar1=float(WSUB), scalar2=None,
                    op0=mybir.AluOpType.is_ge)
                nc.vector.tensor_sub(t1[:], shif[:], slof[:])
                nc.vector.tensor_mul(t1[:], ge[:], t1[:])
                nc.vector.tensor_add(t1[:], slof[:], t1[:])  # chosen s
                nc.vector.scalar_tensor_tensor(
                    jf[:], t1[:], float(WSUB), c2f[:],
                    mybir.AluOpType.mult, mybir.AluOpType.add)
                nc.vector.scalar_tensor_tensor(
                    jf[:], ge[:], float(-WSUB), jf[:],
                    mybir.AluOpType.mult, mybir.AluOpType.add)
                ji = gpool.tile([128, 1], mybir.dt.uint32)
                nc.vector.tensor_copy(ji[:], jf[:])
                nc.gpsimd.indirect_dma_start(
                    Yg[:, rb:rb + 1], None, y_d[:],
                    IndirectOffsetOnAxis(ap=ji[:], axis=0))

            loop_cm = (tc.For_i(0, loop_n, 1) if loop_n
                       else contextlib.nullcontext())
            pend1 = pend2 = None
            with loop_cm:
              for rb in range(n_rb):
                if algo == "countmax":
                    emit_countmax_rb(rb)
                    continue
                if algo == "pairscan":
                    new_pend = emit_pairscan_rb(rb)
                    if pend2 is not None:
                        emit_pairscan_tail(pend2)
                    pend2 = emit_pairscan_mid(pend1) if pend1 else None
                    pend1 = new_pend
                    continue
                if algo == "submax16":
                    emit_submax16_rb(rb, n_act)
                    continue
                if algo == "submax":
                    emit_submax_rb(rb)
                    continue
                # prefix-max of the row is built in two half-row tiles
                halves = [pms_pool.tile([128, half], fp32, name=f"pm{rb}_{h}",
                                        tag="pmh")
                          for h in range(2)]
                for t in range(n_chunk):
                    ps = psum_pool.tile([128, CHUNK], fp32)
                    emit_mms(rb, t, ps)
                    h, tc_ = divmod(t, half_chunks)
                    if parts == "mm":
                        # consume a sliver of PSUM so matmuls are not dead
                        nc.vector.tensor_copy(
                            halves[h][:, tc_ * CHUNK:tc_ * CHUNK + 8],
                            ps[:, 0:8])
                        continue
                    if t == 0:
                        initial = NEGINF
                    elif tc_ == 0:
                        initial = halves[h - 1][:, half - 1:half]
                    else:
                        initial = halves[h][:, tc_ * CHUNK - 1:tc_ * CHUNK]
                    # prefix-max of this chunk, chained to the previous chunk;
                    # data1 is an ignored operand (op1=bypass) shaped like data0.
                    nc.vector.tensor_tensor_scan(
                        halves[h][:, tc_ * CHUNK:(tc_ + 1) * CHUNK],
                        ps[:],
                        dummy[:] if mode == "bf16split" else xb[:, 0:CHUNK],
                        initial,
                        mybir.AluOpType.max,
                        mybir.AluOpType.bypass,
                    )
                if parts != "full":
                    continue
                gt = gpool.tile([128, 1], fp32)
                nc.vector.tensor_copy(gt[:], halves[1][:, half - 1:half])
                # j* = sum_j sign(g - prefix[j]) = #{j: prefix[j] < g};
                # in-place output over the prefix tiles, one accumulator per
                # half, summed later. count_engine picks ACT sign-accum or
                # DVE is_lt-accum (2x mode) per half.
                for h, Jh in ((0, J0), (1, J1)):
                    eng = {"act": "act", "dve": "dve",
                           "split": "act" if h == 0 else "dve"}[count_engine]
                    if eng == "act":
                        nc.scalar.activation(
                            halves[h][:, :],
                            halves[h][:, :],
                            mybir.ActivationFunctionType.Sign,
                            bias=gt[:],
                            scale=-1.0,
                            accum_out=Jh[:, rb:rb + 1],
                        )
                    else:
                        nc.vector.tensor_scalar(
                            out=halves[h][:, :],
                            in0=halves[h][:, :],
                            scalar1=gt[:],
                            scalar2=None,
                            op0=mybir.AluOpType.is_lt,
                            op1=mybir.AluOpType.add,
                            accum_out=Jh[:, rb:rb + 1],
                        )
                # j* for this row-block -> uint32 -> gather y[j*] from DRAM
                ji = gpool.tile([128, 1], mybir.dt.uint32, name=f"ji{rb}",
                                tag="ji")
                nc.vector.scalar_tensor_tensor(
                    ji[:], J0[:, rb:rb + 1], 1.0, J1[:, rb:rb + 1],
                    mybir.AluOpType.mult, mybir.AluOpType.add,
                )
                nc.gpsimd.indirect_dma_start(
                    Yg[:, rb:rb + 1],
                    None,
                    y_d[:],
                    IndirectOffsetOnAxis(ap=ji[:], axis=0),
                )

            if pend2 is not None:
                emit_pairscan_tail(pend2)
            if pend1 is not None:
                emit_pairscan_tail(emit_pairscan_mid(pend1))

            nc.sync.dma_start(out_d[:], Yg[:])

    nc.compile()
    return nc


def prep_inputs(x, xb, y, nq=NQ, m=M, mode="fp32", algo="scan"):
    """Host-side packing. Returns per-core input maps (shared arrays reused)."""
    x = np.asarray(x, dtype=np.float32)
    xb = np.asarray(xb, dtype=np.float32)
    y = np.asarray(y, dtype=np.float32)
    n_chunk = m // CHUNK
    n_rb = nq // RB
    ncores = x.shape[0] // nq
    ytab = np.ascontiguousarray(y.reshape(m, 1))
    in_maps = []

    # pairscan: column q of chunk 2m+s holds ref j = 4096m+2q+s, so the
    # device's chained pair-scan visits pairs {2K, 2K+1} in j order.
    if algo == "pairscan":
        tt = np.arange(n_chunk)[:, None]
        qq = np.arange(CHUNK)[None, :]
        col_perm = (4096 * (tt // 2) + 2 * qq + (tt % 2)).reshape(-1)
    else:
        col_perm = None

    extra = {}
    if algo in ("submax", "submax16", "pairscan", "countmax"):
        xaug = np.empty((K_AUG, m), np.float32)
        xaug[:DIM] = 2.0 * xb.T
        xaug[DIM] = -np.einsum("ij,ij->i", xb, xb)
        extra["xw"] = np.ascontiguousarray(
            xaug.reshape(K_AUG, m // WSUB, WSUB).transpose(1, 0, 2)
            .reshape(m // WSUB, K_AUG * WSUB))

    def add_core_extras(core_maps, c):
        if algo not in ("submax", "submax16", "pairscan", "countmax"):
            return
        arr = np.ones((128, n_rb, K_AUG), np.float32)
        arr[:, :, :DIM] = x[c * nq:(c + 1) * nq].reshape(
            n_rb, RB, DIM).transpose(1, 0, 2)
        core_maps["xqr"] = np.ascontiguousarray(arr.reshape(128, -1))
        core_maps["xw"] = extra["xw"]

    if mode == "bf16split":
        import ml_dtypes

        bf16 = ml_dtypes.bfloat16

        def bf(a):
            return a.astype(bf16).astype(np.float32)

        a = 2.0 * xb.T                      # [16, m]
        ah, al = bf(a), a - bf(a)
        b2 = -np.einsum("ij,ij->i", xb, xb)  # [m]
        b2h, b2l = bf(b2), b2 - bf(b2)
        R = np.zeros((K_SPL, m), np.float32)
        R[0:16] = ah
        R[16] = b2h
        R[17] = b2l
        R[18:34] = ah
        R[34:50] = al
        if col_perm is not None:
            R = R[:, col_perm]
        Rr = R.reshape(K_SPL, n_chunk, TPG, JT)  # u = 2*v + s on axis 2
        XB2 = np.zeros((128, n_chunk * 2, JT), np.float32)
        # strip s handles u in {s, 2+s}; its column block (t*2+v) holds u=2v+s
        for s in range(2):
            XB2[64 * s:64 * s + K_SPL] = Rr[:, :, [s, 2 + s], :].transpose(
                0, 1, 2, 3).reshape(K_SPL, n_chunk * 2, JT)
        xbp = np.ascontiguousarray(
            XB2.reshape(128, n_chunk * 2 * JT)).astype(bf16)

        for c in range(ncores):
            xq = x[c * nq:(c + 1) * nq].T  # [16, nq]
            L = np.zeros((K_SPL, nq), np.float32)
            L[0:16] = bf(xq)
            L[16] = 1.0
            L[17] = 1.0
            L[18:34] = xq - bf(xq)
            L[34:50] = bf(xq)
            XQ2 = np.zeros((128, nq), np.float32)
            for s in range(2):
                XQ2[64 * s:64 * s + K_SPL] = L
            im = {"xq4": XQ2.astype(bf16), "xbp": xbp, "ytab": ytab}
            add_core_extras(im, c)
            in_maps.append(im)
        return in_maps

    # Augmented xb operand: rows 0..15 = 2*xb^T, row 16 = -||xb_j||^2.
    xaug = np.empty((K_AUG, m), np.float32)
    xaug[:DIM] = 2.0 * xb.T
    xaug[DIM] = -np.einsum("ij,ij->i", xb, xb)

    # xbp[32b+k, t*TPG+b, :] = xaug[k, t*CHUNK + b*JT : ... + JT]
    if col_perm is not None:
        xaug = xaug[:, col_perm]
    xa = xaug.reshape(K_AUG, n_chunk, TPG, JT)
    xbp = np.zeros((128, n_chunk * TPG, JT), np.float32)
    for b in range(TPG):
        xbp[32 * b:32 * b + K_AUG, b::TPG, :] = xa[:, :, b, :]
    xbp = np.ascontiguousarray(xbp.reshape(128, n_chunk * TPG * JT))

    for c in range(ncores):
        xq = x[c * nq:(c + 1) * nq]  # [nq, 16]
        xq4 = np.zeros((128, nq), np.float32)
        for b in range(TPG):
            xq4[32 * b:32 * b + DIM] = xq.T
            xq4[32 * b + DIM] = 1.0
        im = {"xq4": xq4, "xbp": xbp, "ytab": ytab}
        add_core_extras(im, c)
        in_maps.append(im)
    return in_maps


def unpack_output(out_np, nq=NQ):
    """[128, n_rb] device layout -> [nq] query order."""
    return np.ascontiguousarray(out_np.T).reshape(nq)


_NC_CACHE = {}
MODE = "bf16split"
ALGO = "pairscan"


def kernel(x, xb, y):
    import concourse.bass_utils as bass_utils

    key = (MODE, ALGO)
    if key not in _NC_CACHE:
        _NC_CACHE[key] = build_nc(mode=MODE, algo=ALGO)
    nc = _NC_CACHE[key]

    in_maps = prep_inputs(x, xb, y, mode=MODE, algo=ALGO)
    res = bass_utils.run_bass_kernel_spmd(nc, in_maps, core_ids=list(range(NCORES)))
    outs = [unpack_output(r["yout"]) for r in res.results]
    return np.concatenate(outs).astype(np.float32)


if __name__ == "__main__":
    # smoke test with random data against numpy reference
    rng = np.random.default_rng(0)
    x = rng.standard_normal((N, DIM), dtype=np.float32)
    xb = rng.standard_normal((M, DIM), dtype=np.float32)
    y = rng.random(M, dtype=np.float32)
    got = kernel(x, xb, y)
    d2 = (np.sum(x * x, 1)[:, None] + np.sum(xb * xb, 1)[None, :]
          - 2.0 * x @ xb.T)
    want = y[np.argmin(d2, axis=1)]
    err = np.abs(got - want)
    print("mismatches:", int((err > 0).sum()), "/", N)



# revision 32
# speedup vs baseline: 1.0200x; 1.0200x over previous
"""KNN retrieval kernel for Trainium2 (8 NeuronCores, data-parallel over queries).

Problem: for each query row x[i] (N=16384, DIM=16), find j* = argmin_j ||xb[j]-x[i]||
over M=16384 reference rows and return y[j*].

Device algorithm (per core, 2048 queries; ALGO="pairscan"):
  ms[i,j] = 2<x_i, xb_j> - ||xb_j||^2   (argmax_j ms == argmin_j dist; the
            ||x_i||^2 term is constant per row and dropped)
  - PE: ms computed as K=50 split-bf16 matmuls (fp32-accurate), 2 j-tiles
    packed into the 128x128 array via 64-row groups, 4 matmuls per 2048-wide
    PSUM chunk.
  - ACT: copies each even chunk PSUM->SBUF (absorbing half the PSUM reads).
  - DVE: chained tensor_tensor_scan(op0=max, op1=max) over (odd-chunk PSUM,
    copied-even-chunk SBUF) ingests TWO score streams per instruction while
    building the row prefix-max P in j order (host packs column q of chunk
    2m+s as ref j = 4096m+2q+s, so scan position K covers pair {2K, 2K+1}).
  - ACT: Sign+accum counts #{k: P[16k+15] < g} (g = last prefix) = the index
    of the first 32-wide j-window containing the row max; exact on ties
    since the prefix is monotone and pairs are j-ordered.
  - GPSIMD: indirect-DMA gathers that window's 32 augmented xb columns; DVE
    re-dots them in exact fp32 and max8/max_index picks the first argmax;
    j* = 32*b* + pos; GPSIMD gathers y[j*].
Host: builds augmented/packed layouts, shards queries 8 ways, reassembles.
"""

import os
import sys

sys.path.insert(0, "/opt/trn_rl_repo")

import numpy as np

N, M, DIM = 16384, 16384, 16
NCORES = 8
NQ = N // NCORES  # queries per core
RB = 128          # row-block (queries per partition block)
JT = 512          # j-tile width (one PSUM bank of fp32)
TPG = 4           # j-tiles packed per PE group (32-row groups)
CHUNK = TPG * JT  # scan chunk width (4 PSUM banks)
K_AUG = 17        # 16 dims + 1 augmentation row
K_SPL = 50        # bf16-split contraction: 16 hi + 2 aug + 16 lo + 16 hi


WSUB = 16         # sub-block width (resolution window, in refs)


def build_nc(nq=NQ, m=M, mode="fp32", loop_n=0, parts="full",
             count_engine="act", algo="scan", n_act=6,
             pair_both_copy=False):
    """Build the per-core Bass module. loop_n>0 wraps the compute in a
    hardware repeat loop (for timing measurement only). parts in
    {"full", "mm", "mmscan"} selects pipeline stages (for perf bisection)."""
    import contextlib
    from contextlib import ExitStack

    import concourse.bacc as bacc
    import concourse.bass as bass
    import concourse.mybir as mybir
    import concourse.tile as tile
    from concourse.bass import IndirectOffsetOnAxis

    fp32 = mybir.dt.float32
    n_rb = nq // RB
    n_chunk = m // CHUNK
    NEGINF = float(np.float32(-3.0e38))

    nc = bacc.Bacc("TRN2", target_bir_lowering=False, debug=False)

    in_dt = mybir.dt.bfloat16 if mode == "bf16split" else fp32
    xb_free = n_chunk * (2 if mode == "bf16split" else TPG) * JT
    xq_d = nc.dram_tensor("xq4", [128, nq], in_dt, kind="ExternalInput")
    xb_d = nc.dram_tensor("xbp", [128, xb_free], in_dt, kind="ExternalInput")
    y_d = nc.dram_tensor("ytab", [m, 1], fp32, kind="ExternalInput")
    out_d = nc.dram_tensor("yout", [128, n_rb], fp32, kind="ExternalOutput")
    if algo in ("submax", "submax16", "pairscan", "countmax"):
        xw_d = nc.dram_tensor("xw", [m // WSUB, K_AUG * WSUB], fp32,
                              kind="ExternalInput")
        xqr_d = nc.dram_tensor("xqr", [128, n_rb * K_AUG], fp32,
                               kind="ExternalInput")

    with tile.TileContext(nc) as tc:
        with ExitStack() as ctx:
            consts = ctx.enter_context(tc.tile_pool(name="consts", bufs=1))
            psum_pool = ctx.enter_context(
                tc.tile_pool(name="ps", bufs=2, space=bass.MemorySpace.PSUM))
            pms_pool = ctx.enter_context(tc.tile_pool(name="pms", bufs=3))
            gpool = ctx.enter_context(tc.tile_pool(name="g", bufs=2))
            outp = ctx.enter_context(tc.tile_pool(name="outp", bufs=1))

            assert n_chunk % 2 == 0
            half_chunks = n_chunk // 2
            half = half_chunks * CHUNK

            xq4 = consts.tile([128, nq], in_dt)
            xb = consts.tile([128, xb_free], in_dt)
            nc.sync.dma_start(xq4[:], xq_d[:])
            nc.sync.dma_start(xb[:], xb_d[:])
            if mode == "bf16split":
                dummy = consts.tile([128, CHUNK], fp32)
                nc.vector.memset(dummy[:], 0.0)

            J0 = outp.tile([128, n_rb], fp32)
            J1 = outp.tile([128, n_rb], fp32)
            Yg = outp.tile([128, n_rb], fp32)
            if parts != "full":
                nc.gpsimd.memset(Yg[:], 0.0)

            def emit_mms(rb, t, ps):
                if mode == "bf16split":
                    # K=50 split-bf16 contraction, 2-way row packing
                    for v in range(2):
                        for s in range(2):
                            u = 2 * v + s
                            nc.tensor.matmul(
                                ps[:, u * JT:(u + 1) * JT],
                                xq4[64 * s:64 * s + K_SPL,
                                    rb * RB:(rb + 1) * RB],
                                xb[64 * s:64 * s + K_SPL,
                                   (t * 2 + v) * JT:(t * 2 + v + 1) * JT],
                                start=True,
                                stop=True,
                                tile_position=(64 * s, 0),
                            )
                else:
                    for b in range(TPG):
                        nc.tensor.matmul(
                            ps[:, b * JT:(b + 1) * JT],
                            xq4[32 * b:32 * b + K_AUG,
                                rb * RB:(rb + 1) * RB],
                            xb[32 * b:32 * b + K_AUG,
                               (t * TPG + b) * JT:(t * TPG + b + 1) * JT],
                            start=True,
                            stop=True,
                            tile_position=(32 * b, 0),
                        )

            if algo in ("submax", "submax16", "pairscan", "countmax"):
                xqr = consts.tile([128, n_rb * K_AUG], fp32)
                nc.sync.dma_start(xqr[:], xqr_d[:])
                smpool = ctx.enter_context(tc.tile_pool(name="sm", bufs=2))
                wpool = ctx.enter_context(tc.tile_pool(name="w", bufs=2))
                nsub = m // WSUB
                cps = CHUNK // WSUB  # sub-blocks per chunk
                WK = WSUB * K_AUG
            if algo in ("submax16", "pairscan"):
                r16pool = ctx.enter_context(tc.tile_pool(name="r16", bufs=2))

            def emit_countmax_rb(rb):
                """submax main loop + count-based tail: per-chunk TR gives
                block maxima SM; a tiny prefix-max scan over SM plus an ACT
                sign-count locate the first block achieving the row max;
                one exact fp32 window re-dot resolves j*."""
                SM = smpool.tile([128, nsub], fp32, name=f"cm{rb}", tag="sm")
                PS = smpool.tile([128, nsub], fp32, name=f"cp{rb}", tag="px")
                for t in range(n_chunk):
                    ps = psum_pool.tile([128, CHUNK], fp32, name=f"p{rb}_{t}",
                                        tag="ps")
                    emit_mms(rb, t, ps)
                    if parts == "mm":
                        nc.vector.tensor_copy(SM[:, t * cps:t * cps + 8],
                                              ps[:, 0:8])
                        continue
                    nc.vector.tensor_reduce(
                        SM[:, t * cps:(t + 1) * cps],
                        ps[:].rearrange("p (s w) -> p s w", w=WSUB),
                        mybir.AxisListType.X,
                        mybir.AluOpType.max,
                    )
                if parts != "full":
                    return
                nc.vector.tensor_tensor_scan(
                    PS[:], SM[:], SM[:], NEGINF,
                    mybir.AluOpType.max, mybir.AluOpType.bypass)
                g = PS[:, nsub - 1:nsub]
                cnt = gpool.tile([128, 1], fp32)
                sj = gpool.tile([128, nsub], fp32, name=f"sj{rb}", tag="sj")
                nc.scalar.activation(
                    sj[:], PS[:], mybir.ActivationFunctionType.Sign,
                    bias=g, scale=-1.0, accum_out=cnt[:])
                bidx = gpool.tile([128, 1], mybir.dt.uint32)
                nc.vector.tensor_copy(bidx[:], cnt[:])
                Wt = wpool.tile([128, WK], fp32)
                nc.gpsimd.indirect_dma_start(
                    Wt[:], None, xw_d[:], IndirectOffsetOnAxis(bidx[:], 0))
                xq_b = (xqr[:, rb * K_AUG:(rb + 1) * K_AUG]
                        .rearrange("p (c k) -> p c k", c=1)
                        .to_broadcast([128, WSUB, K_AUG]))
                Dt = wpool.tile([128, WK], fp32)
                Dd = wpool.tile([128, WSUB], fp32)
                dt_v = Dt[:].rearrange("p (c k) -> p c k", k=K_AUG)
                nc.vector.tensor_tensor(
                    dt_v, Wt[:].rearrange("p (k c) -> p c k", c=WSUB),
                    xq_b, op=mybir.AluOpType.mult)
                nc.vector.tensor_reduce(
                    Dd[:], dt_v, mybir.AxisListType.X, mybir.AluOpType.add)
                cm8 = gpool.tile([128, 8], fp32)
                ci8 = gpool.tile([128, 8], mybir.dt.uint32)
                nc.vector.max(cm8[:], Dd[:])
                nc.vector.max_index(ci8[:], cm8[:], Dd[:])
                c2f = gpool.tile([128, 1], fp32)
                jf = gpool.tile([128, 1], fp32)
                ji = gpool.tile([128, 1], mybir.dt.uint32)
                nc.vector.tensor_copy(c2f[:], ci8[:, 0:1])
                nc.vector.scalar_tensor_tensor(
                    jf[:], cnt[:], float(WSUB), c2f[:],
                    mybir.AluOpType.mult, mybir.AluOpType.add)
                nc.vector.tensor_copy(ji[:], jf[:])
                nc.gpsimd.indirect_dma_start(
                    Yg[:, rb:rb + 1], None, y_d[:],
                    IndirectOffsetOnAxis(ap=ji[:], axis=0))

            def emit_pairscan_rb(rb):
                """Pair-scan: ACT copies even chunks PSUM->SBUF; DVE runs a
                chained tensor_tensor_scan with op0=op1=max over (odd-chunk
                PSUM, copied-even-chunk SBUF) pairs, ingesting TWO score
                streams per cycle while building the row prefix-max in j
                order (host packs column c of chunk 2m+s as ref j =
                4096m+2c+s, so pair index = j//2). The decimated count
                #{P[16k+15] < g} is the 32-wide window holding the first
                max, resolved exactly by one window gather + fp32 re-dot."""
                npair = n_chunk // 2
                P = smpool.tile([128, m // 2], fp32, name=f"pp{rb}",
                                tag="pp")
                for mm_ in range(npair):
                    Ca = r16pool.tile([128, CHUNK], fp32)
                    psA = psum_pool.tile([128, CHUNK], fp32,
                                         name=f"pA{rb}_{mm_}", tag="ps")
                    emit_mms(rb, 2 * mm_, psA)
                    psB = psum_pool.tile([128, CHUNK], fp32,
                                         name=f"pB{rb}_{mm_}", tag="ps")
                    emit_mms(rb, 2 * mm_ + 1, psB)
                    if parts == "mm":
                        nc.vector.tensor_copy(
                            P[:, mm_ * CHUNK:mm_ * CHUNK + 8], psA[:, 0:8])
                        nc.vector.tensor_copy(
                            P[:, mm_ * CHUNK + 8:mm_ * CHUNK + 16],
                            psB[:, 0:8])
                        continue
                    nc.scalar.copy(Ca[:], psA[:])
                    initial = (NEGINF if mm_ == 0
                               else P[:, mm_ * CHUNK - 1:mm_ * CHUNK])
                    if pair_both_copy:
                        Cb = r16pool.tile([128, CHUNK], fp32)
                        nc.scalar.copy(Cb[:], psB[:])
                        data0 = Cb
                    else:
                        data0 = psB
                    nc.vector.tensor_tensor_scan(
                        P[:, mm_ * CHUNK:(mm_ + 1) * CHUNK],
                        data0[:],
                        Ca[:],
                        initial,
                        mybir.AluOpType.max,
                        mybir.AluOpType.max,
                    )
                if parts != "full":
                    return None
                g = P[:, m // 2 - 1:m // 2]
                # window index b* = #{k: P[16k+15] < g} via ACT sign-count
                cnt = gpool.tile([128, 1], fp32)
                npr = WSUB // 2
                sj = gpool.tile([128, (m // 2) // npr], fp32,
                                name=f"sj{rb}", tag="sj")
                nc.scalar.activation(
                    sj[:].rearrange("p (k s) -> p k s", s=1),
                    P[:].rearrange("p (k s) -> p k s",
                                   s=npr)[:, :, npr - 1:npr],
                    mybir.ActivationFunctionType.Sign,
                    bias=g,
                    scale=-1.0,
                    accum_out=cnt[:],
                )
                return (rb, cnt)

            def emit_pairscan_mid(pend):
                rb, cnt = pend
                bidx = gpool.tile([128, 1], mybir.dt.uint32)
                nc.vector.tensor_copy(bidx[:], cnt[:])
                Wt = wpool.tile([128, WK], fp32)
                nc.gpsimd.indirect_dma_start(
                    Wt[:], None, xw_d[:], IndirectOffsetOnAxis(bidx[:], 0))
                return (rb, cnt, Wt)

            def emit_pairscan_tail(pend):
                """Re-dot tail for a previous rb, emitted after the next rb's
                scan chain so the DVE never stalls on the count/gather."""
                rb, cnt, Wt = pend
                # exact fp32 re-dot of the 32-wide window
                xq_b = (xqr[:, rb * K_AUG:(rb + 1) * K_AUG]
                        .rearrange("p (c k) -> p c k", c=1)
                        .to_broadcast([128, WSUB, K_AUG]))
                Dt = wpool.tile([128, WK], fp32)
                Dd = wpool.tile([128, WSUB], fp32)
                dt_v = Dt[:].rearrange("p (c k) -> p c k", k=K_AUG)
                nc.vector.tensor_tensor(
                    dt_v, Wt[:].rearrange("p (k c) -> p c k", c=WSUB),
                    xq_b, op=mybir.AluOpType.mult)
                nc.vector.tensor_reduce(
                    Dd[:], dt_v, mybir.AxisListType.X, mybir.AluOpType.add)
                cm8 = gpool.tile([128, 8], fp32)
                ci8 = gpool.tile([128, 8], mybir.dt.uint32)
                nc.vector.max(cm8[:], Dd[:])
                nc.vector.max_index(ci8[:], cm8[:], Dd[:])
                # j* = 32*b* + pos
                c2f = gpool.tile([128, 1], fp32)
                jf = gpool.tile([128, 1], fp32)
                ji = gpool.tile([128, 1], mybir.dt.uint32)
                nc.vector.tensor_copy(c2f[:], ci8[:, 0:1])
                nc.vector.scalar_tensor_tensor(
                    jf[:], cnt[:], float(WSUB), c2f[:],
                    mybir.AluOpType.mult, mybir.AluOpType.add)
                nc.vector.tensor_copy(ji[:], jf[:])
                nc.gpsimd.indirect_dma_start(
                    Yg[:, rb:rb + 1], None, y_d[:],
                    IndirectOffsetOnAxis(ap=ji[:], axis=0))

            def emit_submax16_rb(rb, n_act):
                """fp16 submax: per chunk, block maxima are computed in fp16
                (monotone rounding keeps the true argmax block tied-at-top;
                the exact fp32 re-dot of the top-2 windows resolves order, and
                max_index returns successive indices for tied values so both
                tied blocks are examined). Chunks < n_act go through an ACT
                copy PSUM->SBUF fp16 + DVE 2x tensor_reduce; the rest use a
                DVE tensor_tensor(max) pairing read straight from PSUM."""
                fp16 = mybir.dt.float16
                cps16 = CHUNK // WSUB  # blocks per chunk (64)
                SM = smpool.tile([128, nsub], fp16, name=f"sm16_{rb}")
                for t in range(n_chunk):
                    ps = psum_pool.tile([128, CHUNK], fp32, name=f"p{rb}_{t}",
                                        tag="ps")
                    emit_mms(rb, t, ps)
                    if parts == "mm":
                        nc.vector.tensor_copy(SM[:, t * cps16:t * cps16 + 8],
                                              ps[:, 0:8])
                        continue
                    if t < n_act:
                        R16 = r16pool.tile([128, CHUNK], fp16)
                        nc.scalar.copy(R16[:], ps[:])
                        nc.vector.tensor_reduce(
                            SM[:, t * cps16:(t + 1) * cps16],
                            R16[:].rearrange("p (s w) -> p s w", w=WSUB),
                            mybir.AxisListType.X,
                            mybir.AluOpType.max,
                        )
                    else:
                        # direct fp32 PSUM reduce (DVE 1x); out cast to fp16
                        nc.vector.tensor_reduce(
                            SM[:, t * cps16:(t + 1) * cps16],
                            ps[:].rearrange("p (s w) -> p s w", w=WSUB),
                            mybir.AxisListType.X,
                            mybir.AluOpType.max,
                        )
                if parts != "full":
                    return
                m8 = gpool.tile([128, 8], fp16)
                i8 = gpool.tile([128, 8], mybir.dt.uint32)
                nc.vector.max(m8[:], SM[:])
                nc.vector.max_index(i8[:], m8[:], SM[:])
                emit_redot_tail(rb, i8)

            def emit_redot_tail(rb, i8):
                """Exact fp32 re-dot of the top-2 candidate windows, then
                j* -> gather y[j*]. Shared by submax/submax16."""
                slo = gpool.tile([128, 1], mybir.dt.uint32)
                shi = gpool.tile([128, 1], mybir.dt.uint32)
                nc.vector.tensor_tensor(slo[:], i8[:, 0:1], i8[:, 1:2],
                                        op=mybir.AluOpType.min)
                nc.vector.tensor_tensor(shi[:], i8[:, 0:1], i8[:, 1:2],
                                        op=mybir.AluOpType.max)
                Wlo = wpool.tile([128, WK], fp32)
                Whi = wpool.tile([128, WK], fp32)
                nc.gpsimd.indirect_dma_start(
                    Wlo[:], None, xw_d[:], IndirectOffsetOnAxis(slo[:], 0))
                nc.gpsimd.indirect_dma_start(
                    Whi[:], None, xw_d[:], IndirectOffsetOnAxis(shi[:], 0))
                # exact fp32 re-dot of the two candidate windows
                xq_b = (xqr[:, rb * K_AUG:(rb + 1) * K_AUG]
                        .rearrange("p (c k) -> p c k", c=1)
                        .to_broadcast([128, WSUB, K_AUG]))
                Dt = wpool.tile([128, 2 * WK], fp32)
                Dd = wpool.tile([128, 2 * WSUB], fp32)
                for wi, Wt in ((0, Wlo), (1, Whi)):
                    dt_v = Dt[:, wi * WK:(wi + 1) * WK].rearrange(
                        "p (c k) -> p c k", k=K_AUG)
                    nc.vector.tensor_tensor(
                        dt_v, Wt[:].rearrange("p (k c) -> p c k", c=WSUB),
                        xq_b, op=mybir.AluOpType.mult)
                    nc.vector.tensor_reduce(
                        Dd[:, wi * WSUB:(wi + 1) * WSUB], dt_v,
                        mybir.AxisListType.X, mybir.AluOpType.add)
                cm8 = gpool.tile([128, 8], fp32)
                ci8 = gpool.tile([128, 8], mybir.dt.uint32)
                nc.vector.max(cm8[:], Dd[:])
                nc.vector.max_index(ci8[:], cm8[:], Dd[:])
                # j* = (c2<W ? slo : shi)*W + c2 mod W, all in fp32
                c2f = gpool.tile([128, 1], fp32)
                slof = gpool.tile([128, 1], fp32)
                shif = gpool.tile([128, 1], fp32)
                ge = gpool.tile([128, 1], fp32)
                t1 = gpool.tile([128, 1], fp32)
                jf = gpool.tile([128, 1], fp32)
                nc.vector.tensor_copy(c2f[:], ci8[:, 0:1])
                nc.vector.tensor_copy(slof[:], slo[:])
                nc.vector.tensor_copy(shif[:], shi[:])
                nc.vector.tensor_scalar(
                    out=ge[:], in0=c2f[:], scalar1=float(WSUB), scalar2=None,
                    op0=mybir.AluOpType.is_ge)
                nc.vector.tensor_sub(t1[:], shif[:], slof[:])
                nc.vector.tensor_mul(t1[:], ge[:], t1[:])
                nc.vector.tensor_add(t1[:], slof[:], t1[:])  # chosen s
                nc.vector.scalar_tensor_tensor(
                    jf[:], t1[:], float(WSUB), c2f[:],
                    mybir.AluOpType.mult, mybir.AluOpType.add)
                nc.vector.scalar_tensor_tensor(
                    jf[:], ge[:], float(-WSUB), jf[:],
                    mybir.AluOpType.mult, mybir.AluOpType.add)
                ji = gpool.tile([128, 1], mybir.dt.uint32)
                nc.vector.tensor_copy(ji[:], jf[:])
                nc.gpsimd.indirect_dma_start(
                    Yg[:, rb:rb + 1], None, y_d[:],
                    IndirectOffsetOnAxis(ap=ji[:], axis=0))

            def emit_submax_rb(rb):
                SM = smpool.tile([128, nsub], fp32)
                for t in range(n_chunk):
                    ps = psum_pool.tile([128, CHUNK], fp32, name=f"p{rb}_{t}",
                                        tag="ps")
                    emit_mms(rb, t, ps)
                    if parts == "mm":
                        nc.vector.tensor_copy(SM[:, t * cps:t * cps + 8],
                                              ps[:, 0:8])
                        continue
                    nc.vector.tensor_reduce(
                        SM[:, t * cps:(t + 1) * cps],
                        ps[:].rearrange("p (s w) -> p s w", w=WSUB),
                        mybir.AxisListType.X,
                        mybir.AluOpType.max,
                    )
                if parts != "full":
                    return
                # top-2 sub-blocks (value-ranked, then j-ordered)
                m8 = gpool.tile([128, 8], fp32)
                i8 = gpool.tile([128, 8], mybir.dt.uint32)
                nc.vector.max(m8[:], SM[:])
                nc.vector.max_index(i8[:], m8[:], SM[:])
                slo = gpool.tile([128, 1], mybir.dt.uint32)
                shi = gpool.tile([128, 1], mybir.dt.uint32)
                nc.vector.tensor_tensor(slo[:], i8[:, 0:1], i8[:, 1:2],
                                        op=mybir.AluOpType.min)
                nc.vector.tensor_tensor(shi[:], i8[:, 0:1], i8[:, 1:2],
                                        op=mybir.AluOpType.max)
                Wlo = wpool.tile([128, WK], fp32)
                Whi = wpool.tile([128, WK], fp32)
                nc.gpsimd.indirect_dma_start(
                    Wlo[:], None, xw_d[:], IndirectOffsetOnAxis(slo[:], 0))
                nc.gpsimd.indirect_dma_start(
                    Whi[:], None, xw_d[:], IndirectOffsetOnAxis(shi[:], 0))
                # exact fp32 re-dot of the two candidate windows
                xq_b = (xqr[:, rb * K_AUG:(rb + 1) * K_AUG]
                        .rearrange("p (c k) -> p c k", c=1)
                        .to_broadcast([128, WSUB, K_AUG]))
                Dt = wpool.tile([128, 2 * WK], fp32)
                Dd = wpool.tile([128, 2 * WSUB], fp32)
                for wi, Wt in ((0, Wlo), (1, Whi)):
                    dt_v = Dt[:, wi * WK:(wi + 1) * WK].rearrange(
                        "p (c k) -> p c k", k=K_AUG)
                    nc.vector.tensor_tensor(
                        dt_v, Wt[:].rearrange("p (k c) -> p c k", c=WSUB),
                        xq_b, op=mybir.AluOpType.mult)
                    nc.vector.tensor_reduce(
                        Dd[:, wi * WSUB:(wi + 1) * WSUB], dt_v,
                        mybir.AxisListType.X, mybir.AluOpType.add)
                cm8 = gpool.tile([128, 8], fp32)
                ci8 = gpool.tile([128, 8], mybir.dt.uint32)
                nc.vector.max(cm8[:], Dd[:])
                nc.vector.max_index(ci8[:], cm8[:], Dd[:])
                # j* = (c2<W ? slo : shi)*W + c2 mod W, all in fp32
                c2f = gpool.tile([128, 1], fp32)
                slof = gpool.tile([128, 1], fp32)
                shif = gpool.tile([128, 1], fp32)
                ge = gpool.tile([128, 1], fp32)
                t1 = gpool.tile([128, 1], fp32)
                jf = gpool.tile([128, 1], fp32)
                nc.vector.tensor_copy(c2f[:], ci8[:, 0:1])
                nc.vector.tensor_copy(slof[:], slo[:])
                nc.vector.tensor_copy(shif[:], shi[:])
                nc.vector.tensor_scalar(
                    out=ge[:], in0=c2f[:], scalar1=float(WSUB), scalar2=None,
                    op0=mybir.AluOpType.is_ge)
                nc.vector.tensor_sub(t1[:], shif[:], slof[:])
                nc.vector.tensor_mul(t1[:], ge[:], t1[:])
                nc.vector.tensor_add(t1[:], slof[:], t1[:])  # chosen s
                nc.vector.scalar_tensor_tensor(
                    jf[:], t1[:], float(WSUB), c2f[:],
                    mybir.AluOpType.mult, mybir.AluOpType.add)
                nc.vector.scalar_tensor_tensor(
                    jf[:], ge[:], float(-WSUB), jf[:],
                    mybir.AluOpType.mult, mybir.AluOpType.add)
                ji = gpool.tile([128, 1], mybir.dt.uint32)
                nc.vector.tensor_copy(ji[:], jf[:])
                nc.gpsimd.indirect_dma_start(
                    Yg[:, rb:rb + 1], None, y_d[:],
                    IndirectOffsetOnAxis(ap=ji[:], axis=0))

            loop_cm = (tc.For_i(0, loop_n, 1) if loop_n
                       else contextlib.nullcontext())
            pend = None
            with loop_cm:
              for rb in range(n_rb):
                if algo == "countmax":
                    emit_countmax_rb(rb)
                    continue
                if algo == "pairscan":
                    new_pend = emit_pairscan_rb(rb)
                    if pend is not None:
                        emit_pairscan_tail(pend)
                    pend = (emit_pairscan_mid(new_pend)
                            if new_pend is not None else None)
                    continue
                if algo == "submax16":
                    emit_submax16_rb(rb, n_act)
                    continue
                if algo == "submax":
                    emit_submax_rb(rb)
                    continue
                # prefix-max of the row is built in two half-row tiles
                halves = [pms_pool.tile([128, half], fp32, name=f"pm{rb}_{h}",
                                        tag="pmh")
                          for h in range(2)]
                for t in range(n_chunk):
                    ps = psum_pool.tile([128, CHUNK], fp32)
                    emit_mms(rb, t, ps)
                    h, tc_ = divmod(t, half_chunks)
                    if parts == "mm":
                        # consume a sliver of PSUM so matmuls are not dead
                        nc.vector.tensor_copy(
                            halves[h][:, tc_ * CHUNK:tc_ * CHUNK + 8],
                            ps[:, 0:8])
                        continue
                    if t == 0:
                        initial = NEGINF
                    elif tc_ == 0:
                        initial = halves[h - 1][:, half - 1:half]
                    else:
                        initial = halves[h][:, tc_ * CHUNK - 1:tc_ * CHUNK]
                    # prefix-max of this chunk, chained to the previous chunk;
                    # data1 is an ignored operand (op1=bypass) shaped like data0.
                    nc.vector.tensor_tensor_scan(
                        halves[h][:, tc_ * CHUNK:(tc_ + 1) * CHUNK],
                        ps[:],
                        dummy[:] if mode == "bf16split" else xb[:, 0:CHUNK],
                        initial,
                        mybir.AluOpType.max,
                        mybir.AluOpType.bypass,
                    )
                if parts != "full":
                    continue
                gt = gpool.tile([128, 1], fp32)
                nc.vector.tensor_copy(gt[:], halves[1][:, half - 1:half])
                # j* = sum_j sign(g - prefix[j]) = #{j: prefix[j] < g};
                # in-place output over the prefix tiles, one accumulator per
                # half, summed later. count_engine picks ACT sign-accum or
                # DVE is_lt-accum (2x mode) per half.
                for h, Jh in ((0, J0), (1, J1)):
                    eng = {"act": "act", "dve": "dve",
                           "split": "act" if h == 0 else "dve"}[count_engine]
                    if eng == "act":
                        nc.scalar.activation(
                            halves[h][:, :],
                            halves[h][:, :],
                            mybir.ActivationFunctionType.Sign,
                            bias=gt[:],
                            scale=-1.0,
                            accum_out=Jh[:, rb:rb + 1],
                        )
                    else:
                        nc.vector.tensor_scalar(
                            out=halves[h][:, :],
                            in0=halves[h][:, :],
                            scalar1=gt[:],
                            scalar2=None,
                            op0=mybir.AluOpType.is_lt,
                            op1=mybir.AluOpType.add,
                            accum_out=Jh[:, rb:rb + 1],
                        )
                # j* for this row-block -> uint32 -> gather y[j*] from DRAM
                ji = gpool.tile([128, 1], mybir.dt.uint32, name=f"ji{rb}",
                                tag="ji")
                nc.vector.scalar_tensor_tensor(
                    ji[:], J0[:, rb:rb + 1], 1.0, J1[:, rb:rb + 1],
                    mybir.AluOpType.mult, mybir.AluOpType.add,
                )
                nc.gpsimd.indirect_dma_start(
                    Yg[:, rb:rb + 1],
                    None,
                    y_d[:],
                    IndirectOffsetOnAxis(ap=ji[:], axis=0),
                )

            if pend is not None:
                emit_pairscan_tail(pend)

            nc.sync.dma_start(out_d[:], Yg[:])

    nc.compile()
    return nc


def prep_inputs(x, xb, y, nq=NQ, m=M, mode="fp32", algo="scan"):
    """Host-side packing. Returns per-core input maps (shared arrays reused)."""
    x = np.asarray(x, dtype=np.float32)
    xb = np.asarray(xb, dtype=np.float32)
    y = np.asarray(y, dtype=np.float32)
    n_chunk = m // CHUNK
    n_rb = nq // RB
    ncores = x.shape[0] // nq
    ytab = np.ascontiguousarray(y.reshape(m, 1))
    in_maps = []

    # pairscan: column q of chunk 2m+s holds ref j = 4096m+2q+s, so the
    # device's chained pair-scan visits pairs {2K, 2K+1} in j order.
    if algo == "pairscan":
        tt = np.arange(n_chunk)[:, None]
        qq = np.arange(CHUNK)[None, :]
        col_perm = (4096 * (tt // 2) + 2 * qq + (tt % 2)).reshape(-1)
    else:
        col_perm = None

    extra = {}
    if algo in ("submax", "submax16", "pairscan", "countmax"):
        xaug = np.empty((K_AUG, m), np.float32)
        xaug[:DIM] = 2.0 * xb.T
        xaug[DIM] = -np.einsum("ij,ij->i", xb, xb)
        extra["xw"] = np.ascontiguousarray(
            xaug.reshape(K_AUG, m // WSUB, WSUB).transpose(1, 0, 2)
            .reshape(m // WSUB, K_AUG * WSUB))

    def add_core_extras(core_maps, c):
        if algo not in ("submax", "submax16", "pairscan", "countmax"):
            return
        arr = np.ones((128, n_rb, K_AUG), np.float32)
        arr[:, :, :DIM] = x[c * nq:(c + 1) * nq].reshape(
            n_rb, RB, DIM).transpose(1, 0, 2)
        core_maps["xqr"] = np.ascontiguousarray(arr.reshape(128, -1))
        core_maps["xw"] = extra["xw"]

    if mode == "bf16split":
        import ml_dtypes

        bf16 = ml_dtypes.bfloat16

        def bf(a):
            return a.astype(bf16).astype(np.float32)

        a = 2.0 * xb.T                      # [16, m]
        ah, al = bf(a), a - bf(a)
        b2 = -np.einsum("ij,ij->i", xb, xb)  # [m]
        b2h, b2l = bf(b2), b2 - bf(b2)
        R = np.zeros((K_SPL, m), np.float32)
        R[0:16] = ah
        R[16] = b2h
        R[17] = b2l
        R[18:34] = ah
        R[34:50] = al
        if col_perm is not None:
            R = R[:, col_perm]
        Rr = R.reshape(K_SPL, n_chunk, TPG, JT)  # u = 2*v + s on axis 2
        XB2 = np.zeros((128, n_chunk * 2, JT), np.float32)
        # strip s handles u in {s, 2+s}; its column block (t*2+v) holds u=2v+s
        for s in range(2):
            XB2[64 * s:64 * s + K_SPL] = Rr[:, :, [s, 2 + s], :].transpose(
                0, 1, 2, 3).reshape(K_SPL, n_chunk * 2, JT)
        xbp = np.ascontiguousarray(
            XB2.reshape(128, n_chunk * 2 * JT)).astype(bf16)

        for c in range(ncores):
            xq = x[c * nq:(c + 1) * nq].T  # [16, nq]
            L = np.zeros((K_SPL, nq), np.float32)
            L[0:16] = bf(xq)
            L[16] = 1.0
            L[17] = 1.0
            L[18:34] = xq - bf(xq)
            L[34:50] = bf(xq)
            XQ2 = np.zeros((128, nq), np.float32)
            for s in range(2):
                XQ2[64 * s:64 * s + K_SPL] = L
            im = {"xq4": XQ2.astype(bf16), "xbp": xbp, "ytab": ytab}
            add_core_extras(im, c)
            in_maps.append(im)
        return in_maps

    # Augmented xb operand: rows 0..15 = 2*xb^T, row 16 = -||xb_j||^2.
    xaug = np.empty((K_AUG, m), np.float32)
    xaug[:DIM] = 2.0 * xb.T
    xaug[DIM] = -np.einsum("ij,ij->i", xb, xb)

    # xbp[32b+k, t*TPG+b, :] = xaug[k, t*CHUNK + b*JT : ... + JT]
    if col_perm is not None:
        xaug = xaug[:, col_perm]
    xa = xaug.reshape(K_AUG, n_chunk, TPG, JT)
    xbp = np.zeros((128, n_chunk * TPG, JT), np.float32)
    for b in range(TPG):
        xbp[32 * b:32 * b + K_AUG, b::TPG, :] = xa[:, :, b, :]
    xbp = np.ascontiguousarray(xbp.reshape(128, n_chunk * TPG * JT))

    for c in range(ncores):
        xq = x[c * nq:(c + 1) * nq]  # [nq, 16]
        xq4 = np.zeros((128, nq), np.float32)
        for b in range(TPG):
            xq4[32 * b:32 * b + DIM] = xq.T
            xq4[32 * b + DIM] = 1.0
        im = {"xq4": xq4, "xbp": xbp, "ytab": ytab}
        add_core_extras(im, c)
        in_maps.append(im)
    return in_maps


def unpack_output(out_np, nq=NQ):
    """[128, n_rb] device layout -> [nq] query order."""
    return np.ascontiguousarray(out_np.T).reshape(nq)


_NC_CACHE = {}
MODE = "bf16split"
ALGO = "pairscan"


def kernel(x, xb, y):
    import concourse.bass_utils as bass_utils

    key = (MODE, ALGO)
    if key not in _NC_CACHE:
        _NC_CACHE[key] = build_nc(mode=MODE, algo=ALGO)
    nc = _NC_CACHE[key]

    in_maps = prep_inputs(x, xb, y, mode=MODE, algo=ALGO)
    res = bass_utils.run_bass_kernel_spmd(nc, in_maps, core_ids=list(range(NCORES)))
    outs = [unpack_output(r["yout"]) for r in res.results]
    return np.concatenate(outs).astype(np.float32)


if __name__ == "__main__":
    # smoke test with random data against numpy reference
    rng = np.random.default_rng(0)
    x = rng.standard_normal((N, DIM), dtype=np.float32)
    xb = rng.standard_normal((M, DIM), dtype=np.float32)
    y = rng.random(M, dtype=np.float32)
    got = kernel(x, xb, y)
    d2 = (np.sum(x * x, 1)[:, None] + np.sum(xb * xb, 1)[None, :]
          - 2.0 * x @ xb.T)
    want = y[np.argmin(d2, axis=1)]
    err = np.abs(got - want)
    print("mismatches:", int((err > 0).sum()), "/", N)

